# revision 13
# baseline (speedup 1.0000x reference)
"""GRU kernel for Trainium2, 8 NeuronCores.

Strategy:
  Phase A (projections): row-split 8 ways (time-major rows t*64+b). Each core
    computes xrzn = x @ [W_ir|W_iz|W_in] + biases for its 4096 rows, all 3072
    gate columns (fp32r matmuls, x.T stationary / W streaming).
  AllGather: one ncfw AllGather replicates xrzn [32768, 3072] to every core.
  Phase B (recurrence, 512 sequential steps): tensor-parallel across core
    pairs (i, i^1). Even cores compute gate/hidden columns [0:512) of each
    gate ("half 0"), odd cores half 1. Per step: h.T-stationary fp32r matmuls
    accumulate h @ W_h + (xr|xz|b_hn) into PSUM via an identity matmul;
    sigmoid/tanh + elementwise produce h_new[:, half]; PE transposes h_new
    into h.T chunks; one remote_dma_broadcast sends them to the pair peer.
  Gate column layout (global): [r0 z0 n0 | r1 z1 n1], each block 512 wide.
Outputs: (output [T,B,HID], h_T [B,HID]) as float32, matching the reference.
"""
import sys, os, time
os.environ.setdefault("NEURON_SCRATCHPAD_PAGE_SIZE", "1024")
sys.path.insert(0, '/opt/trn_rl_repo')
import numpy as np

B, T, IN, HID = 64, 512, 1024, 1024
NC = 8
HALF = HID // 2          # 512
GW = 3 * HALF            # 1536 per-core gate cols in recurrence
ROWS = B * T             # 32768 (time-major: row = t*B + b)
RPC = ROWS // 4          # 8192 rows per core in phase A (4 cores per parity group)
KCH = IN // 128          # 8 contraction chunks

_CACHE = {}


def _build_nc(nsteps=T):
    import concourse.bass as bass
    import concourse.bacc as bacc
    import concourse.mybir as mybir
    import concourse.tile as tile
    from concourse.tile_rust import add_dep_helper

    f32 = mybir.dt.float32
    f32r = mybir.dt.float32r

    nc = bacc.Bacc("TRN2", target_bir_lowering=False, debug=False,
                   enable_asserts=False, num_devices=NC)

    # ---------------- DRAM tensors (per-core inputs) ----------------
    xT_d = nc.dram_tensor("xT", [IN, RPC], f32r, kind="ExternalInput")       # x.T slice (my rows quarter)
    Wg_d = nc.dram_tensor("Wg", [IN, GW], f32r, kind="ExternalInput")         # input-side weights, my half cols
    bg_d = nc.dram_tensor("bg", [128, GW], f32, kind="ExternalInput")         # bias rows broadcast
    Wh_d = nc.dram_tensor("Wh", [HID, GW], f32r, kind="ExternalInput")       # recurrent weights, my half cols, rows permuted (own half first)
    bhn_d = nc.dram_tensor("bhn", [B, HALF], f32r, kind="ExternalInput")     # b_hn (my half) broadcast over batch
    eye_d = nc.dram_tensor("eye", [B, B], f32r, kind="ExternalInput")        # I_64
    hout_d = nc.dram_tensor("hout", [nsteps, B, HALF], f32, kind="ExternalOutput")

    # internal DRAM: my xrzn slice + gathered
    xr_my_d = nc.dram_tensor("xr_my", [RPC, GW], f32r)
    xr_all_d = nc.dram_tensor("xr_all", [2 * ROWS, GW], f32r)

    # persistent SBUF
    send = nc.alloc_sbuf_tensor("send", [128, 2 * 4 * B], f32r).ap()   # [par][4 chunks x 64] my h.T chunks
    recv = nc.alloc_sbuf_tensor("recv", [128, 2 * 4 * B], f32r).ap()   # peer's h.T chunks
    Wh_sb = nc.alloc_sbuf_tensor("Wh_sb", [128, KCH * GW], f32r).ap()  # resident recurrent weights
    bhn_sb = nc.alloc_sbuf_tensor("bhn_sb", [B, HALF], f32r).ap()
    eye_sb = nc.alloc_sbuf_tensor("eye_sb", [B, B], f32r).ap()
    hprev = nc.alloc_sbuf_tensor("hprev", [B, 2 * HALF], f32r).ap()    # my h half, per parity

    rsem = nc.alloc_semaphore("rsem")   # peer arrivals: +2 per step
    lsem = nc.alloc_semaphore("lsem")   # my send drained: +16 per step
    msem = nc.alloc_semaphore("msem")   # send buffer ready (DVE nop): +1 per step

    deferred = []
    last_eng = {}

    def chain(inst):
        eng = inst.ins.engine
        prev = last_eng.get(eng)
        if prev is not None:
            add_dep_helper(inst.ins, prev.ins, sync=False, reason="step order")
        last_eng[eng] = inst
        return inst

    def dwait(inst, sem, val):
        chain(inst)
        deferred.append((inst, sem, val))
        return inst

    W512 = 512

    with tile.TileContext(nc) as tc:
        # =================== Phase A: projections ===================
        with tc.tile_pool(name="pa_w", bufs=1) as pw, \
             tc.tile_pool(name="pa_x", bufs=2) as px, \
             tc.tile_pool(name="pa_o", bufs=2) as po, \
             tc.tile_pool(name="pa_ps", bufs=2, space="PSUM") as pps:
            bg_t = pw.tile([128, GW], f32)
            nc.sync.dma_start(bg_t[:], bg_d[:])
            Wg_t = pw.tile([128, KCH * GW], f32r, tag="wg")
            Wg_r = Wg_d[:].rearrange("(c p) n -> c p n", p=128)
            for c in range(KCH):
                nc.sync.dma_start(Wg_t[:, c * GW:(c + 1) * GW], Wg_r[c])
            for m in range(RPC // 128):
                xt = px.tile([128, KCH * 128], f32r, tag="xt")
                xT_r = xT_d[:, m * 128:(m + 1) * 128].rearrange("(c p) r -> c p r", p=128)
                for c in range(KCH):
                    nc.sync.dma_start(xt[:, c * 128:(c + 1) * 128], xT_r[c])
                ps = pps.tile([128, GW], f32, tag="ps")
                for w in range(3):
                    for kc in range(KCH):
                        nc.tensor.matmul(ps[:, w * W512:(w + 1) * W512],
                                         xt[:, kc * 128:(kc + 1) * 128],
                                         Wg_t[:, kc * GW + w * W512: kc * GW + (w + 1) * W512],
                                         start=(kc == 0), stop=(kc == KCH - 1))
                ot = po.tile([128, GW], f32r, tag="ot")
                for w in range(3):
                    nc.vector.tensor_add(ot[:, w * W512:(w + 1) * W512],
                                         ps[:, w * W512:(w + 1) * W512],
                                         bg_t[:, w * W512:(w + 1) * W512])
                nc.sync.dma_start(xr_my_d[m * 128:(m + 1) * 128, :], ot[:])

        # =================== AllGather xrzn (within parity group) ===================
        nc.gpsimd.collective_compute(
            "AllGather", mybir.AluOpType.bypass,
            replica_groups=[list(range(NC))],
            ins=[xr_my_d[:].bitcast(f32).opt()],
            outs=[xr_all_d[:].bitcast(f32).opt()],
        )

        # =================== Phase B: recurrence ===================
        with tc.tile_pool(name="pb_x", bufs=4) as pbx, \
             tc.tile_pool(name="pb_e", bufs=2) as pbe, \
             tc.tile_pool(name="pb_ps", bufs=2, space="PSUM") as pbps, \
             tc.tile_pool(name="pb_tp", bufs=2, space="PSUM") as pbtp:

            halfsel = nc.sync.partition_id() & 1
            Wh_r = Wh_d[:].rearrange("(c p) n -> c p n", p=128)
            for c in range(KCH):
                nc.sync.dma_start(Wh_sb[:, c * GW:(c + 1) * GW], Wh_r[c])
            nc.sync.dma_start(bhn_sb[:], bhn_d[:])
            nc.sync.dma_start(eye_sb[:], eye_d[:])
            nc.vector.memset(send[:, 0:4 * B].bitcast(f32), 0.0)
            nc.vector.memset(recv[:, 0:4 * B].bitcast(f32), 0.0)
            nc.vector.memset(hprev[:, 0:HALF].bitcast(f32), 0.0)

            for t in range(nsteps):
                par = t % 2
                npar = (t + 1) % 2
                sA = send[:, par * 4 * B:(par + 1) * 4 * B]       # my h.T chunks (K rows 0-511 of permuted Wh)
                sB = recv[:, par * 4 * B:(par + 1) * 4 * B]       # peer's chunks (K rows 512-1023)
                hp = hprev[:, par * HALF:(par + 1) * HALF]

                # xrzn tile for this step: my-half block of quarter q
                q = t // 128
                xt = pbx.tile([B, GW], f32r, tag="xt")
                row0 = q * RPC + t * B - q * RPC + q * RPC  # = t*B within quarter? compute directly below
                base = (2 * q) * RPC + (t - q * 128) * B
                xr_r = xr_all_d[:].rearrange("(blk r) n -> blk r n", blk=2 * 4)
                nc.sync.dma_start(xt[:], xr_r[bass.ds(2 * q + halfsel, 1), (t - q * 128) * B:(t - q * 128 + 1) * B, :].opt())

                ps = pbps.tile([B, GW], f32, tag="ps")
                first_mm = {}
                for w in range(3):
                    for kc in range(KCH):
                        stat = sA[:, (kc % 4) * B:(kc % 4 + 1) * B] if kc < 4 else sB[:, (kc - 4) * B:(kc - 3) * B]
                        mm = nc.tensor.matmul(ps[:, w * W512:(w + 1) * W512], stat,
                                              Wh_sb[:, kc * GW + w * W512: kc * GW + (w + 1) * W512],
                                              start=(kc == 0), stop=False)
                        if kc == 0 and w == 0:
                            dwait(mm, rsem, 2 * t)   # peer round t-1 arrived
                        else:
                            chain(mm)
                    rhs = xt[:, w * W512:(w + 1) * W512] if w < 2 else bhn_sb[:]
                    chain(nc.tensor.matmul(ps[:, w * W512:(w + 1) * W512], eye_sb[:], rhs,
                                           start=False, stop=True))

                # elementwise
                rz = pbe.tile([B, 2 * W512], f32, tag="rz")
                chain(nc.scalar.activation(rz[:], ps[:, 0:2 * W512],
                                           mybir.ActivationFunctionType.Sigmoid))
                q = pbe.tile([B, W512], f32, tag="q")
                chain(nc.scalar.activation(q[:], rz[:, W512:2 * W512],
                                           mybir.ActivationFunctionType.Copy, bias=1.0, scale=-1.0))
                t1 = pbe.tile([B, W512], f32, tag="t1")
                chain(nc.vector.tensor_mul(t1[:], rz[:, 0:W512], ps[:, 2 * W512:3 * W512]))
                chain(nc.vector.tensor_add(t1[:], t1[:], xt[:, 2 * W512:3 * W512].bitcast(f32)))
                n_t = pbe.tile([B, W512], f32, tag="n_t")
                chain(nc.scalar.activation(n_t[:], t1[:], mybir.ActivationFunctionType.Tanh))
                p_t = pbe.tile([B, W512], f32, tag="p_t")
                chain(nc.gpsimd.tensor_mul(p_t[:], rz[:, W512:2 * W512], hp.bitcast(f32)))
                # prep this round's broadcast (desc-gen on Pool, off critical path)
                if t + 1 < nsteps:
                    chain(nc.gpsimd.remote_dma_broadcast(
                        recv[:, npar * 4 * B:(npar + 1) * 4 * B],
                        send[:, npar * 4 * B:(npar + 1) * 4 * B],
                        rsem, lsem, rdests=[None, (0, 1)] + [None] * 6))
                h5 = hprev[:, npar * HALF:(npar + 1) * HALF]
                t5 = pbe.tile([B, W512], f32, tag="t5")
                chain(nc.vector.tensor_mul(t5[:], q[:], n_t[:]))
                chain(nc.vector.tensor_add(h5, t5[:], p_t[:]))  # DVE rounds to f32r on write
                # output
                nc.sync.dma_start(hout_d[t], h5.bitcast(f32))

                # transpose h5 -> 4 h.T chunks into send[npar]
                if t + 1 < nsteps:
                    for c in range(4):
                        tp = pbtp.tile([128, B], f32r, tag="tp")
                        chain(nc.tensor.transpose(tp[:], h5[:, c * 128:(c + 1) * 128], eye_sb[:]))
                        cp = nc.vector.tensor_copy(send[:, (npar * 4 + c) * B:(npar * 4 + c + 1) * B], tp[:])
                        dwait(cp, lsem, 16 * t)   # my round t-1 bytes drained
                    mn = nc.vector.nop(nofuse=True)
                    add_dep_helper(mn.ins, cp.ins, sync=False, reason="msem")
                    mn.then_inc(msem, 1)
                    tr = nc.gpsimd.trigger_dma(count=None)
                    tr._wait_ge(msem, t + 1)
                    chain(tr)

    for inst, sem, val in deferred:
        if val > 0:
            inst.wait_op(sem, val, "sem-ge", check=False)
    nc.compile()
    return nc


def _host_prep(inputs, core):
    """Build the per-core input map (numpy) for core index `core`."""
    x = inputs["inputs"]  # [B, T, IN] f32
    half = core & 1
    hs = slice(half * HALF, (half + 1) * HALF)

    key = "_xT"
    if key not in _CACHE:
        _CACHE[key] = np.ascontiguousarray(
            x.transpose(2, 1, 0).reshape(IN, T * B).astype(np.float32))
    xT = _CACHE[key]

    Wg = np.empty((IN, GW), np.float32)
    bgv = np.empty(GW, np.float32)
    Wg[:, 0:HALF] = inputs["W_ir"][:, hs]
    Wg[:, HALF:2 * HALF] = inputs["W_iz"][:, hs]
    Wg[:, 2 * HALF:3 * HALF] = inputs["W_in"][:, hs]
    bgv[0:HALF] = (inputs["b_ir"] + inputs["b_hr"])[hs]
    bgv[HALF:2 * HALF] = (inputs["b_iz"] + inputs["b_hz"])[hs]
    bgv[2 * HALF:3 * HALF] = inputs["b_in"][hs]
    bg = np.tile(bgv[None, :], (128, 1)).astype(np.float32)

    # recurrent weights: cols = my half [r|z|n]; rows permuted own-half first
    Wh = np.empty((HID, GW), np.float32)
    Wh[:, 0:HALF] = inputs["W_hr"][:, hs]
    Wh[:, HALF:2 * HALF] = inputs["W_hz"][:, hs]
    Wh[:, 2 * HALF:3 * HALF] = inputs["W_hn"][:, hs]
    row_perm = np.concatenate([np.arange(half * HALF, (half + 1) * HALF),
                               np.arange((1 - half) * HALF, (2 - half) * HALF)])
    Wh = np.ascontiguousarray(Wh[row_perm])

    bhn = np.tile(inputs["b_hn"][hs][None, :], (B, 1)).astype(np.float32)
    eye = np.eye(B, dtype=np.float32)

    return dict(
        xT=np.ascontiguousarray(xT[:, (core // 2) * RPC:(core // 2 + 1) * RPC]),
        Wg=Wg, bg=bg, Wh=Wh, bhn=bhn, eye=eye,
    )


def _run(nc, in_maps, n_timed=0):
    import jax
    from jax.sharding import Mesh, PartitionSpec
    from jax.experimental.shard_map import shard_map
    import concourse.mybir as mybir
    from concourse.bass2jax import install_neuronx_cc_hook, _bass_exec_p, partition_id_tensor

    install_neuronx_cc_hook()
    partition_name = nc.partition_id_tensor.name if nc.partition_id_tensor else None
    in_names, out_names, out_avals, zero_outs = [], [], [], []
    for alloc in nc.m.functions[0].allocations:
        if not isinstance(alloc, mybir.MemoryLocationSet):
            continue
        name = alloc.memorylocations[0].name
        if alloc.kind == "ExternalInput":
            if name != partition_name:
                in_names.append(name)
        elif alloc.kind == "ExternalOutput":
            out_names.append(name)
            shape = tuple(alloc.tensor_shape)
            dtype = mybir.dt.np(alloc.dtype)
            out_avals.append(jax.core.ShapedArray(shape, dtype))
            zero_outs.append(np.zeros(shape, dtype))
    n_params, n_outs = len(in_names), len(out_avals)
    in_names_full = in_names + out_names + ([partition_name] if partition_name else [])

    def _body(*args):
        operands = list(args)
        if partition_name is not None:
            operands.append(partition_id_tensor())
        outs = _bass_exec_p.bind(
            *operands, out_avals=tuple(out_avals), in_names=tuple(in_names_full),
            out_names=tuple(out_names), lowering_input_output_aliases=(),
            sim_require_finite=True, sim_require_nnan=True, nc=nc)
        return tuple(outs)

    devices = jax.devices()[:NC]
    mesh = Mesh(np.asarray(devices), ("core",))
    sharded = jax.jit(shard_map(_body, mesh=mesh,
                                in_specs=(PartitionSpec("core"),) * (n_params + n_outs),
                                out_specs=(PartitionSpec("core"),) * n_outs,
                                check_rep=False))
    sharding = jax.sharding.NamedSharding(mesh, PartitionSpec("core"))
    concat_in = [jax.device_put(
        np.concatenate([np.asarray(in_maps[c][in_names[i]], dtype=np.float32) for c in range(NC)], axis=0),
        sharding) for i in range(n_params)]
    concat_zeros = [jax.device_put(np.zeros((NC * z.shape[0], *z.shape[1:]), z.dtype), sharding)
                    for z in zero_outs]
    jax.block_until_ready(concat_in)
    out_arrs = sharded(*concat_in, *concat_zeros)
    jax.block_until_ready(out_arrs)
    times = []
    for _ in range(n_timed):
        t0 = time.time()
        out_arrs = sharded(*concat_in, *concat_zeros)
        jax.block_until_ready(out_arrs)
        times.append(time.time() - t0)
    results = [
        {name: np.asarray(out_arrs[i]).reshape(NC, *out_avals[i].shape)[c]
         for i, name in enumerate(out_names)}
        for c in range(NC)
    ]
    return results, times


def kernel(**inputs):
    if "nc" not in _CACHE:
        _CACHE["nc"] = _build_nc(T)
    nc = _CACHE["nc"]
    for k in ("_xT", "_Wg", "_bg"):
        _CACHE.pop(k, None)
    in_maps = [_host_prep(inputs, c) for c in range(NC)]
    results, _ = _run(nc, in_maps)
    out = np.empty((T, B, HID), np.float32)
    out[:, :, 0:HALF] = results[0]["hout"]
    out[:, :, HALF:HID] = results[1]["hout"]
    return out, out[-1].copy()
